# revision 1
# baseline (speedup 1.0000x reference)
"""LoRA MultiheadAttention on 8 NeuronCores (Bass/Tile).

Sharding: 32 (batch, head) attention slices -> 4 heads x 1 batch per core.
Cores 0-3 take batch 0, cores 4-7 batch 1; core c handles heads
(c%4)*4 .. (c%4)*4+3, i.e. a contiguous 256-wide slice of the head dims.

Per-core math (all big matmuls bf16 on PE, fp32 PSUM accumulate):
  xaT   (1152, 2048) = [X^T; ones-row; zero pad]  (bias via ones row)
  qkT   = wqk^T-slices @ X  -> Q^T, K^T in (head-dim, T) layout
          (q pre-scaled by 1/sqrt(hd); LoRA K accumulated into same PSUM)
  V     = X @ Wv-slice (natural (T, dv) layout, per-head 65-wide blocks with
          a ones column -> PV matmul emits the softmax denominator for free)
  S^T   = K^T.T-slices @ Q^T  (tj on partitions, ti free)  [K=64 contraction]
  P^T   = exp(S^T)  on ACT  (no max-subtraction: |scores| <~ 3 by construction)
  O^T   = V_aug.T @ P^T  accumulated over tj; row 64 = denom
  norm  : denom row broadcast across 64 partitions via K=1 PE matmul with a
          ones column, reciprocal on DVE, multiply -> normalized O^T (bf16)
  out   = O^T.T @ out_w-slice^T  (T, 1024) fp32 partial, summed on host.

b_v is folded into the V matmul ones-row bias; out_b added on host.
"""

import sys

sys.path.insert(0, "/opt/trn_rl_repo")

import math
from contextlib import ExitStack

import ml_dtypes
import numpy as np

import concourse.bass as bass
import concourse.tile as tile
from concourse import bacc
from concourse import mybir
from concourse.bass_utils import run_bass_kernel_spmd

BF16 = ml_dtypes.bfloat16
F32 = mybir.dt.float32
BF = mybir.dt.bfloat16

T = 2048
D = 1024
H = 16
HD = 64
R = 16
BSZ = 2
SCALE = 16.0
NCORES = 8
HPC = 4  # heads per core
CD = HPC * HD  # 256 head dims per core
VW = HD + 1  # V block width per head (ones column appended)
KPAD = 1152  # 1024 X rows + 1 ones row, padded to 9 k-tiles of 128
NKT = KPAD // 128
P = 128
NTT = T // P  # 16 row tiles
HF = T // 2  # 1024: ti processed in two halves


def build_nc():
    nc = bass.Bass()
    xa = nc.dram_tensor("xa", [KPAD, T], BF, kind="ExternalInput")
    wqk = nc.dram_tensor("wqk", [KPAD, 2 * CD], BF, kind="ExternalInput")
    wv = nc.dram_tensor("wv", [KPAD, HPC * VW], BF, kind="ExternalInput")
    ab = nc.dram_tensor("ab", [KPAD, 3 * R], BF, kind="ExternalInput")
    kbm = nc.dram_tensor("kbm", [R, CD], BF, kind="ExternalInput")
    vbm = nc.dram_tensor("vbm", [R, HPC * VW], BF, kind="ExternalInput")
    wo = nc.dram_tensor("wo", [CD, D], BF, kind="ExternalInput")
    out = nc.dram_tensor("out", [T, D], F32, kind="ExternalOutput")

    with tile.TileContext(nc) as tc, ExitStack() as ctx:
        singles = ctx.enter_context(tc.tile_pool(name="singles", bufs=1))

        xa_t = [singles.tile([P, T], BF, name=f"xa{i}", tag=f"xa{i}") for i in range(NKT)]
        wqk_t = [singles.tile([P, 2 * CD], BF, name=f"wqk{i}", tag=f"wqk{i}") for i in range(NKT)]
        wv_t = [singles.tile([P, HPC * VW], BF, name=f"wv{i}", tag=f"wv{i}") for i in range(NKT)]
        ab_t = [singles.tile([P, 3 * R], BF, name=f"ab{i}", tag=f"ab{i}") for i in range(NKT)]
        kb_t = singles.tile([R, CD], BF, tag="kb")
        vb_t = singles.tile([R, HPC * VW], BF, tag="vb")
        wo_t = [singles.tile([P, D], BF, name=f"wo{i}", tag=f"wo{i}") for i in range(2)]
        for i in range(NKT):
            nc.sync.dma_start(out=xa_t[i], in_=xa[i * P : (i + 1) * P, :])
            nc.sync.dma_start(out=wqk_t[i], in_=wqk[i * P : (i + 1) * P, :])
            nc.sync.dma_start(out=wv_t[i], in_=wv[i * P : (i + 1) * P, :])
            nc.sync.dma_start(out=ab_t[i], in_=ab[i * P : (i + 1) * P, :])
        nc.sync.dma_start(out=kb_t, in_=kbm[:, :])
        nc.sync.dma_start(out=vb_t, in_=vbm[:, :])
        for i in range(2):
            nc.sync.dma_start(out=wo_t[i], in_=wo[i * P : (i + 1) * P, :])

        ones_t = singles.tile([1, HD], F32, tag="ones")
        nc.vector.memset(ones_t, 1.0)

        qk_sb = [singles.tile([P, T], BF, name=f"qk{i}", tag=f"qk{i}") for i in range(4)]
        ak_sb = singles.tile([R, T], BF, tag="ak")
        av_sb = singles.tile([R, T], BF, tag="av")
        v_sb = [singles.tile([P, HPC * VW], BF, name=f"v{i}", tag=f"v{i}") for i in range(NTT)]
        oT_sb = [singles.tile([P, T], BF, name=f"oT{i}", tag=f"oT{i}") for i in range(2)]

        # Phase A: A_kv^T = [k_a; v_a] @ X   (32, T)
        with tc.tile_pool(name="pA", bufs=2, space="PSUM") as pA:
            for ch in range(4):
                cs = slice(ch * 512, (ch + 1) * 512)
                pa = pA.tile([3 * R, 512], F32, tag="pa")
                for kt in range(8):  # ab rows >= 1024 are zero; skip 9th tile
                    nc.tensor.matmul(
                        pa,
                        lhsT=ab_t[kt],
                        rhs=xa_t[kt][:, cs],
                        start=(kt == 0),
                        stop=(kt == 7),
                    )
                nc.vector.tensor_copy(ak_sb[:, cs], pa[0:R, :])
                nc.vector.tensor_copy(av_sb[:, cs], pa[2 * R : 3 * R, :])

        # Phase B: Q^T, K^T (4 m-tiles of 128) with LoRA-K accumulated
        with tc.tile_pool(name="pB", bufs=3, space="PSUM") as pB:
            for m in range(4):
                for ch in range(4):
                    cs = slice(ch * 512, (ch + 1) * 512)
                    pq = pB.tile([P, 512], F32, tag="pq")
                    for kt in range(NKT):
                        nc.tensor.matmul(
                            pq,
                            lhsT=wqk_t[kt][:, m * P : (m + 1) * P],
                            rhs=xa_t[kt][:, cs],
                            start=(kt == 0),
                            stop=(kt == NKT - 1 and m < 2),
                        )
                    if m >= 2:
                        nc.tensor.matmul(
                            pq,
                            lhsT=kb_t[:, (m - 2) * P : (m - 1) * P],
                            rhs=ak_sb[:, cs],
                            start=False,
                            stop=True,
                        )
                    nc.vector.tensor_copy(qk_sb[m][:, cs], pq)

        # Phase C: V natural (T, 4*65) with ones cols + b_v via ones-row, LoRA-V
        with tc.tile_pool(name="pC", bufs=3, space="PSUM") as pC:
            for mt in range(NTT):
                ms = slice(mt * P, (mt + 1) * P)
                pv = pC.tile([P, HPC * VW], F32, tag="pv")
                for kt in range(NKT):
                    nc.tensor.matmul(
                        pv,
                        lhsT=xa_t[kt][:, ms],
                        rhs=wv_t[kt],
                        start=(kt == 0),
                        stop=False,
                    )
                nc.tensor.matmul(
                    pv, lhsT=av_sb[:, ms], rhs=vb_t, start=False, stop=True
                )
                nc.vector.tensor_copy(v_sb[mt], pv)

        # Phase D+E: attention units (half-outer, head-inner), software-
        # pipelined normalize (unit i's normalize emitted after unit i+1's
        # matmuls so PE never stalls at unit boundaries), denominator
        # broadcast via DRAM round-trip DMA (stride-0 partition read) instead
        # of a PE matmul, and half-0 out-proj overlapped with half-1 attention.
        with (
            tc.tile_pool(name="pS", bufs=3, space="PSUM") as pS,
            tc.tile_pool(name="pO", bufs=2, space="PSUM") as pO,
            tc.tile_pool(name="pE", bufs=1, space="PSUM") as pE,
            tc.tile_pool(name="pP", bufs=6) as pP,
            tc.tile_pool(name="pN", bufs=2) as pN,
            tc.tile_pool(name="pD", bufs=2, space="DRAM") as pD,
            tc.tile_pool(name="pOut", bufs=3) as pOut,
        ):
            def emit_unit(half, h):
                qT = qk_sb[h // 2][(h % 2) * HD : (h % 2) * HD + HD, :]
                kT = qk_sb[2 + h // 2][(h % 2) * HD : (h % 2) * HD + HD, :]
                po = pO.tile([VW, HF], F32, tag="po", name=f"po_{half}_{h}")
                pts = {}

                def emit_pv(tjp):
                    for q2 in range(2):
                        nc.tensor.matmul(
                            po[:, q2 * 512 : (q2 + 1) * 512],
                            lhsT=v_sb[tjp][:, h * VW : (h + 1) * VW],
                            rhs=pts.pop((tjp, q2)),
                            start=(tjp == 0),
                            stop=(tjp == NTT - 1),
                        )

                # PV shifted one tj behind S so exp(tj) overlaps S(tj+1) and
                # PE never waits on ACT (keeps the >=3us continuous-execution
                # window that promotes PE to the full 2.4 GHz p-state).
                for tj in range(NTT):
                    for q2 in range(2):
                        qs = slice(half * HF + q2 * 512, half * HF + (q2 + 1) * 512)
                        ps = pS.tile([P, 512], F32, tag="spsum", name=f"ps_{half}_{h}_{tj}_{q2}")
                        nc.tensor.matmul(
                            ps,
                            lhsT=kT[:, tj * P : (tj + 1) * P],
                            rhs=qT[:, qs],
                            start=True,
                            stop=True,
                        )
                        pt = pP.tile([P, 512], BF, tag="pt", name=f"pt_{half}_{h}_{tj}_{q2}")
                        nc.scalar.activation(pt, ps, mybir.ActivationFunctionType.Exp)
                        pts[(tj, q2)] = pt
                    if tj > 0:
                        emit_pv(tj - 1)
                emit_pv(NTT - 1)
                return po

            def emit_norm(half, h, po):
                hs = slice(half * HF, (half + 1) * HF)
                den = pN.tile([1, HF], F32, tag="den", name=f"den_{half}_{h}")
                nc.vector.tensor_copy(den, po[HD:VW, :])
                dr = pD.tile([1, HF], F32, tag="dr", name=f"dr_{half}_{h}")
                nc.sync.dma_start(out=dr, in_=den)
                den64 = pN.tile([HD, HF], F32, tag="den64", name=f"den64_{half}_{h}")
                nc.sync.dma_start(
                    out=den64,
                    in_=bass.AP(tensor=dr.tensor, offset=dr.offset, ap=[[0, HD], [1, HF]]),
                )
                rec = pN.tile([HD, HF], F32, tag="rec", name=f"rec_{half}_{h}")
                nc.vector.reciprocal(rec, den64)
                nc.vector.tensor_mul(
                    oT_sb[h // 2][(h % 2) * HD : (h % 2) * HD + HD, hs],
                    po[0:HD, :],
                    rec,
                )

            def emit_outproj(half):
                for mt in range(half * 8, (half + 1) * 8):
                    ms = slice(mt * P, (mt + 1) * P)
                    ob = pOut.tile([P, D], F32, tag="ob", name=f"ob_{mt}")
                    for ch in range(2):
                        cs = slice(ch * 512, (ch + 1) * 512)
                        po2 = pE.tile([P, 512], F32, tag="po2", name=f"po2_{mt}_{ch}")
                        for kt2 in range(2):
                            nc.tensor.matmul(
                                po2,
                                lhsT=oT_sb[kt2][:, ms],
                                rhs=wo_t[kt2][:, cs],
                                start=(kt2 == 0),
                                stop=(kt2 == 1),
                            )
                        nc.vector.tensor_copy(ob[:, cs], po2)
                    nc.sync.dma_start(out=out[ms, :], in_=ob)

            units = [(half, h) for half in range(2) for h in range(HPC)]
            prev = None
            for i, (half, h) in enumerate(units):
                po = emit_unit(half, h)
                if prev is not None:
                    emit_norm(prev[0], prev[1], prev[2])
                    if i == 4:
                        emit_outproj(0)
                prev = (half, h, po)
            emit_norm(prev[0], prev[1], prev[2])
            emit_outproj(1)

    # bass.Bass's finalize skips Bacc's wait-splitting passes; walrus allows
    # at most 1 sync wait per instruction (2 for event semaphores), so run
    # just those two passes here.
    import bass_rust as _bass_rust

    _bass_rust.move_matmul_waits_to_ldweights(nc.m)
    _bass_rust.generate_event_semaphores(nc)
    return nc


def prepare_in_maps(inputs):
    q = np.asarray(inputs["query"], np.float32)
    ipw = np.asarray(inputs["in_proj_weight"], np.float32)
    ipb = np.asarray(inputs["in_proj_bias"], np.float32)
    out_w = np.asarray(inputs["out_w"], np.float32)
    k_a = np.asarray(inputs["k_a"], np.float32)
    k_b = np.asarray(inputs["k_b"], np.float32)
    v_a = np.asarray(inputs["v_a"], np.float32)
    v_b = np.asarray(inputs["v_b"], np.float32)
    qscale = 1.0 / math.sqrt(HD)
    sl = SCALE / R

    in_maps = []
    for c in range(NCORES):
        bb = c // 4
        s = (c % 4) * CD
        e = s + CD
        X = q[:, bb, :]

        xa = np.zeros((KPAD, T), np.float32)
        xa[:D] = X.T
        xa[D] = 1.0

        wqk = np.zeros((KPAD, 2 * CD), np.float32)
        wqk[:D, :CD] = ipw[s:e].T * qscale
        wqk[D, :CD] = ipb[s:e] * qscale
        wqk[:D, CD:] = ipw[D + s : D + e].T
        wqk[D, CD:] = ipb[D + s : D + e]

        wv = np.zeros((KPAD, HPC * VW), np.float32)
        for j in range(HPC):
            wv[:D, j * VW : j * VW + HD] = ipw[2 * D + s + j * HD : 2 * D + s + (j + 1) * HD].T
            wv[D, j * VW : j * VW + HD] = ipb[2 * D + s + j * HD : 2 * D + s + (j + 1) * HD]
            wv[D, j * VW + HD] = 1.0

        ab = np.zeros((KPAD, 3 * R), np.float32)
        ab[:D, :R] = k_a.T
        ab[:D, 2 * R :] = v_a.T

        kbm = k_b[:, s:e] * sl

        vbm = np.zeros((R, HPC * VW), np.float32)
        for j in range(HPC):
            vbm[:, j * VW : j * VW + HD] = v_b[:, s + j * HD : s + (j + 1) * HD] * sl

        wo = out_w[:, s:e].T

        in_maps.append(
            {
                "xa": xa.astype(BF16),
                "wqk": wqk.astype(BF16),
                "wv": wv.astype(BF16),
                "ab": ab.astype(BF16),
                "kbm": kbm.astype(BF16),
                "vbm": vbm.astype(BF16),
                "wo": wo.astype(BF16),
            }
        )
    return in_maps


def assemble_output(inputs, results):
    out_b = np.asarray(inputs["out_b"], np.float32)
    out = np.zeros((T, BSZ, D), np.float32)
    for c in range(NCORES):
        out[:, c // 4, :] += results[c]["out"]
    out += out_b[None, None, :]
    return out


def kernel(**inputs):
    nc = build_nc()
    in_maps = prepare_in_maps(inputs)
    res = run_bass_kernel_spmd(nc, in_maps, core_ids=list(range(NCORES)))
    return assemble_output(inputs, res.results)



# revision 6
# speedup vs baseline: 1.4185x; 1.4185x over previous
"""LoRA MultiheadAttention on 8 NeuronCores (Bass/Tile).

Sharding: 32 (batch, head) attention slices -> 4 heads x 1 batch per core.
Cores 0-3 take batch 0, cores 4-7 batch 1; core c handles heads
(c%4)*4 .. (c%4)*4+3, i.e. a contiguous 256-wide slice of the head dims.

Host-side weight prep (pure algebra, no runtime input compute):
  - LoRA folded into the projections: Wk_eff = Wk + s*k_b^T k_a (same for V),
    so the device kernel is a plain attention kernel.
  - K bias dropped (softmax row-shift invariance), V bias folded into out_b
    (softmax rows sum to 1), Q bias applied on the ACT engine during PSUM
    evacuation (per-partition bias AP).  KPAD therefore = 1024 (8 k-tiles,
    no ones-row).

Per-core device schedule (all matmuls bf16 on PE, fp32 PSUM):
  B     : Q^T, K^T (head-dim, T layout), 4 m-tiles x 4 chunks x 8 k-tiles
  C     : V natural (T, 4*65) with a ones column per head (PV emits the
          softmax denominator for free)
  attn  : 8 units (2 query halves x 4 heads).  Per unit, per tj-tile:
          S^T = K^T.T @ Q^T into a [128,1024] 2-bank PSUM tile, ONE wide
          exp on ACT ([128,1024] -> bf16 SBUF).  PV runs one full UNIT
          behind (unit i's loop interleaves PV of unit i-1), so ACT paces
          the phase and PE always has independent work.
  norm  : denominator row -> DRAM round-trip broadcast (stride-0 partition
          read) -> reciprocal -> multiply into oT_sb (bf16)
  OP    : out = O^T.T @ wo after attention; PSUM evacuation split
          between ACT and DVE; fp32 partials summed on host.
"""

import sys

sys.path.insert(0, "/opt/trn_rl_repo")

import math
from contextlib import ExitStack

import ml_dtypes
import numpy as np

import concourse.bass as bass
import concourse.tile as tile
from concourse import mybir
from concourse.bass_utils import run_bass_kernel_spmd

BF16 = ml_dtypes.bfloat16
F32 = mybir.dt.float32
BF = mybir.dt.bfloat16

T = 2048
D = 1024
H = 16
HD = 64
R = 16
BSZ = 2
SCALE = 16.0
NCORES = 8
HPC = 4  # heads per core
CD = HPC * HD  # 256 head dims per core
VW = HD + 1  # V block width per head (ones column appended)
KPAD = 1024  # contraction dim (no ones row; biases handled elsewhere)
NKT = KPAD // 128  # 8 k-tiles
P = 128
NTT = T // P  # 16 row tiles
HF = T // 2  # 1024: ti processed in two halves


def build_nc():
    nc = bass.Bass()
    xa = nc.dram_tensor("xa", [KPAD, T], BF, kind="ExternalInput")
    wqk = nc.dram_tensor("wqk", [KPAD, 2 * CD], BF, kind="ExternalInput")
    wv = nc.dram_tensor("wv", [KPAD, HPC * VW], BF, kind="ExternalInput")
    qb = nc.dram_tensor("qb", [P, 2], F32, kind="ExternalInput")
    wo = nc.dram_tensor("wo", [CD, D], BF, kind="ExternalInput")
    out = nc.dram_tensor("out", [T, D], F32, kind="ExternalOutput")

    with tile.TileContext(nc) as tc, ExitStack() as ctx:
        singles = ctx.enter_context(tc.tile_pool(name="singles", bufs=1))

        xa_t = [singles.tile([P, T], BF, name=f"xa{i}", tag=f"xa{i}") for i in range(NKT)]
        wqk_t = [singles.tile([P, 2 * CD], BF, name=f"wqk{i}", tag=f"wqk{i}") for i in range(NKT)]
        wv_t = [singles.tile([P, HPC * VW], BF, name=f"wv{i}", tag=f"wv{i}") for i in range(NKT)]
        qb_t = singles.tile([P, 2], F32, tag="qb")
        wo_t = [singles.tile([P, D], BF, name=f"wo{i}", tag=f"wo{i}") for i in range(2)]
        # interleave big input loads so phase B's k-chain tracks DMA arrival
        for i in range(NKT):
            nc.sync.dma_start(out=xa_t[i], in_=xa[i * P : (i + 1) * P, :])
            nc.sync.dma_start(out=wqk_t[i], in_=wqk[i * P : (i + 1) * P, :])
        for i in range(NKT):
            nc.sync.dma_start(out=wv_t[i], in_=wv[i * P : (i + 1) * P, :])
        nc.sync.dma_start(out=qb_t, in_=qb[:, :])
        for i in range(2):
            nc.sync.dma_start(out=wo_t[i], in_=wo[i * P : (i + 1) * P, :])

        qk_sb = [singles.tile([P, T], BF, name=f"qk{i}", tag=f"qk{i}") for i in range(4)]
        v_sb = [singles.tile([P, HPC * VW], BF, name=f"v{i}", tag=f"v{i}") for i in range(NTT)]
        oT_sb = [singles.tile([P, T], BF, name=f"oT{i}", tag=f"oT{i}") for i in range(2)]

        # Phase B: Q^T, K^T (4 m-tiles of 128); Q bias added on ACT during evac
        with tc.tile_pool(name="pB", bufs=3, space="PSUM") as pB:
            for m in range(4):
                for ch in range(4):
                    cs = slice(ch * 512, (ch + 1) * 512)
                    pq = pB.tile([P, 512], F32, tag="pq")
                    for kt in range(NKT):
                        nc.tensor.matmul(
                            pq,
                            lhsT=wqk_t[kt][:, m * P : (m + 1) * P],
                            rhs=xa_t[kt][:, cs],
                            start=(kt == 0),
                            stop=(kt == NKT - 1),
                        )
                    if m < 2:
                        nc.scalar.add(qk_sb[m][:, cs], pq, qb_t[:, m : m + 1])
                    else:
                        nc.scalar.copy(qk_sb[m][:, cs], pq)

        # Attention units: unit i's loop emits S+exp for unit i and PV for
        # unit i-1; ACT (wide exp) paces the phase.  Phase C (V projection)
        # overlaps unit 0's exps; its PSUM banks are released before the
        # first PV output tile is needed (pC closes before pO opens).
        with (
            tc.tile_pool(name="pS", bufs=2, space="PSUM") as pS,
            tc.tile_pool(name="pP", bufs=20) as pP,
            tc.tile_pool(name="pN", bufs=2) as pN,
            tc.tile_pool(name="pD", bufs=2, space="DRAM") as pD,
        ):
            units = [(half, h) for half in range(2) for h in range(HPC)]
            pts = {}  # (unit_idx, tj) -> pt tile
            pos = {}  # unit_idx -> po tile

            def emit_s_exp(i, tj):
                half, h = units[i]
                qT = qk_sb[h // 2][(h % 2) * HD : (h % 2) * HD + HD, :]
                kT = qk_sb[2 + h // 2][(h % 2) * HD : (h % 2) * HD + HD, :]
                ps = pS.tile([P, 2 * 512], F32, tag="ps", name=f"ps_{i}_{tj}")
                for q2 in range(2):
                    qs = slice(half * HF + q2 * 512, half * HF + (q2 + 1) * 512)
                    nc.tensor.matmul(
                        ps[:, q2 * 512 : (q2 + 1) * 512],
                        lhsT=kT[:, tj * P : (tj + 1) * P],
                        rhs=qT[:, qs],
                        start=True,
                        stop=True,
                    )
                pt = pP.tile([P, 2 * 512], BF, tag="pt", name=f"pt_{i}_{tj}")
                nc.scalar.activation(pt, ps, mybir.ActivationFunctionType.Exp)
                pts[(i, tj)] = pt

            def emit_pv(i, tj, pO):
                half, h = units[i]
                po = pos[i]
                pt = pts.pop((i, tj))
                for q2 in range(2):
                    nc.tensor.matmul(
                        po[:, q2 * 512 : (q2 + 1) * 512],
                        lhsT=v_sb[tj][:, h * VW : (h + 1) * VW],
                        rhs=pt[:, q2 * 512 : (q2 + 1) * 512],
                        start=(tj == 0),
                        stop=(tj == NTT - 1),
                    )

            def emit_norm(i):
                half, h = units[i]
                po = pos.pop(i)
                hs = slice(half * HF, (half + 1) * HF)
                den = pN.tile([1, HF], F32, tag="den", name=f"den_{i}")
                nc.vector.tensor_copy(den, po[HD:VW, :])
                dr = pD.tile([1, HF], F32, tag="dr", name=f"dr_{i}")
                nc.sync.dma_start(out=dr, in_=den)
                den64 = pN.tile([HD, HF], F32, tag="den64", name=f"den64_{i}")
                nc.sync.dma_start(
                    out=den64,
                    in_=bass.AP(tensor=dr.tensor, offset=dr.offset, ap=[[0, HD], [1, HF]]),
                )
                rec = pN.tile([HD, HF], F32, tag="rec", name=f"rec_{i}")
                nc.vector.reciprocal(rec, den64)
                nc.vector.tensor_mul(
                    oT_sb[h // 2][(h % 2) * HD : (h % 2) * HD + HD, hs],
                    po[0:HD, :],
                    rec,
                )

            # unit 0: S+exp only; phase C overlaps unit 0's exps
            with tc.tile_pool(name="pC", bufs=3, space="PSUM") as pC:
                for tj in range(NTT):
                    emit_s_exp(0, tj)
                for mt in range(NTT):
                    ms = slice(mt * P, (mt + 1) * P)
                    pv = pC.tile([P, HPC * VW], F32, tag="pv", name=f"pv_{mt}")
                    for kt in range(NKT):
                        nc.tensor.matmul(
                            pv,
                            lhsT=xa_t[kt][:, ms],
                            rhs=wv_t[kt],
                            start=(kt == 0),
                            stop=(kt == NKT - 1),
                        )
                    nc.scalar.copy(v_sb[mt], pv)
                    # ones columns (denominator trick): constant 1, not
                    # producible by X @ wv -- set after the evac overwrite
                    nc.vector.memset(v_sb[mt][:, HD::VW], 1.0)

            with tc.tile_pool(name="pO", bufs=2, space="PSUM") as pO:
                # units 1..7: S+exp of unit i, PV of unit i-1 interleaved
                for i in range(1, 8):
                    pos[i - 1] = pO.tile([VW, HF], F32, tag="po", name=f"po_{i - 1}")
                    for tj in range(NTT):
                        emit_s_exp(i, tj)
                        emit_pv(i - 1, tj, pO)
                    emit_norm(i - 1)
                # tail: PV + norm of unit 7
                pos[7] = pO.tile([VW, HF], F32, tag="po", name="po_7")
                for tj in range(NTT):
                    emit_pv(7, tj, pO)
                emit_norm(7)

        # Out-projection; evacuation split between ACT and DVE
        with (
            tc.tile_pool(name="pE", bufs=2, space="PSUM") as pE,
            tc.tile_pool(name="pOut", bufs=3) as pOut,
        ):
            for mt in range(NTT):
                ms = slice(mt * P, (mt + 1) * P)
                ob = pOut.tile([P, D], F32, tag="ob", name=f"ob_{mt}")
                for ch in range(2):
                    cs = slice(ch * 512, (ch + 1) * 512)
                    po2 = pE.tile([P, 512], F32, tag="po2", name=f"po2_{mt}_{ch}")
                    for kt2 in range(2):
                        nc.tensor.matmul(
                            po2,
                            lhsT=oT_sb[kt2][:, ms],
                            rhs=wo_t[kt2][:, cs],
                            start=(kt2 == 0),
                            stop=(kt2 == 1),
                        )
                    if ch == 0:
                        nc.scalar.copy(ob[:, cs], po2)
                    else:
                        nc.vector.tensor_copy(ob[:, cs], po2)
                nc.sync.dma_start(out=out[ms, :], in_=ob)

    # bass.Bass's finalize skips Bacc's wait-splitting passes; walrus allows
    # at most 1 sync wait per instruction (2 for event semaphores), so run
    # just those two passes here.
    import bass_rust as _bass_rust

    _bass_rust.move_matmul_waits_to_ldweights(nc.m)
    _bass_rust.generate_event_semaphores(nc)
    return nc


def prepare_in_maps(inputs):
    q = np.asarray(inputs["query"], np.float32)
    ipw = np.asarray(inputs["in_proj_weight"], np.float32)
    ipb = np.asarray(inputs["in_proj_bias"], np.float32)
    k_a = np.asarray(inputs["k_a"], np.float32)
    k_b = np.asarray(inputs["k_b"], np.float32)
    v_a = np.asarray(inputs["v_a"], np.float32)
    v_b = np.asarray(inputs["v_b"], np.float32)
    out_w = np.asarray(inputs["out_w"], np.float32)
    qscale = 1.0 / math.sqrt(HD)
    sl = SCALE / R

    # fold LoRA into the K/V projection weights (pure weight algebra)
    wk_eff = ipw[D : 2 * D] + sl * (k_b.T @ k_a)  # (D, D)
    wv_eff = ipw[2 * D : 3 * D] + sl * (v_b.T @ v_a)  # (D, D)

    in_maps = []
    for c in range(NCORES):
        bb = c // 4
        s = (c % 4) * CD
        e = s + CD
        X = q[:, bb, :]

        xa = np.ascontiguousarray(X.T)

        wqk = np.empty((KPAD, 2 * CD), np.float32)
        wqk[:, :CD] = ipw[s:e].T * qscale
        wqk[:, CD:] = wk_eff[s:e].T

        # V weights; ones columns stay 0 here (set to 1 in v_sb on device)
        wv = np.zeros((KPAD, HPC * VW), np.float32)
        for j in range(HPC):
            wv[:, j * VW : j * VW + HD] = wv_eff[s + j * HD : s + (j + 1) * HD].T

        qbias = (ipb[s:e] * qscale).astype(np.float32).reshape(2, P).T  # (128, 2)
        qbias = np.ascontiguousarray(qbias)

        wo = out_w[:, s:e].T

        in_maps.append(
            {
                "xa": xa.astype(BF16),
                "wqk": wqk.astype(BF16),
                "wv": wv.astype(BF16),
                "qb": qbias,
                "wo": wo.astype(BF16),
            }
        )
    return in_maps


def assemble_output(inputs, results):
    out_b = np.asarray(inputs["out_b"], np.float32)
    ipb = np.asarray(inputs["in_proj_bias"], np.float32)
    out_w = np.asarray(inputs["out_w"], np.float32)
    # V bias folded through softmax (rows sum to 1) and out-projection
    out_b_eff = out_b + ipb[2 * D : 3 * D] @ out_w.T
    out = np.zeros((T, BSZ, D), np.float32)
    for c in range(NCORES):
        out[:, c // 4, :] += results[c]["out"]
    out += out_b_eff[None, None, :]
    return out


def kernel(**inputs):
    nc = build_nc()
    in_maps = prepare_in_maps(inputs)
    res = run_bass_kernel_spmd(nc, in_maps, core_ids=list(range(NCORES)))
    return assemble_output(inputs, res.results)
